# revision 22
# baseline (speedup 1.0000x reference)
"""Trainium2 Bass kernel for nn_Attr_Relation_Net (gnn_message_passing).

Computation per edge e (E=500k edges):
    m_i   = known_mask[obs_mask_idx[e]]          # [64] binary
    m_j   = 1 - onehot(attr_idx[e])              # [64] binary
    m_JI  = softmax(m_i * m_j)                   # closed form: (1+(e-1)s)/D
    m_JI  = gelu(gelu(m_JI @ W_rm1 + b_rm1) @ W_rm2 + b_rm2)
    a     = gelu((fea_corr[attr_idx[e]] * m_JI) @ W_rr + b_rr)
    out   = gelu((obs_embs[obs_idx[e]] * a) @ W_rc + b_rc)

v3 design (vs the fp32 v1 baseline at ~950us simulated):
  * all matmul/elementwise traffic in bf16 (fp32 matmuls cost 4 cycles/row
    on TRN2 vs 1 for bf16); tables pre-cast on the host.
  * obs_embs rows arrive FEAT-MAJOR via dma_gather transpose=True (256B
    bf16 rows, single_packet=False -- the single-packet transpose ucode
    path crashes this hardware), eliminating the per-chunk PE transposes
    of obs_h. Gathers stay at 1024 indices (ucode descriptor-ring cap).
  * binary-softmax in closed form with the (e-1) factor pre-scaled into
    the known_mask table: q = (s+1)*rcol, s = km_scaled[m]*(iota!=attr),
    rcol = 1/(64+sum(s)).
  * F=64 stages run pair-stacked on partitions (chunks 0-3 -> rows 0:63,
    chunks 4-7 -> rows 64:127), halving DVE/ACT cost of the F stages.
    Contractions over a 64-row half use full-K matmuls with zero-padded
    weight copies (row-group tile_position crashes this hardware; the
    col-group form used by the phi_rm layer-2 pair works).
  * fea_corr rows via block-diag matmul of a one-hot built from a
    host-replicated attr broadcast table (SBUF 4x-mode compare, no PSUM).
  * final matmul keeps edges on the free dim (out = W_rc^T @ (obs_h*a)):
    output lands feat-major [H, e] and DMAs contiguously as bf16; the
    host transposes/scatters/upcasts while unsharding.
  * 3-stage software pipelining (emission skewed across pairs) so each
    engine's FIFO interleaves three pairs; PSUM tags sized for a
    one-pair reuse lag inside the 8-bank budget. Index tables load first,
    weight/bias consts load under the priming gathers, and a 2-group
    gather lookahead keeps the DMA engines gap-free in steady state.
"""

import math

import numpy as np
import ml_dtypes

import concourse.bacc as bacc_mod
import concourse.mybir as mybir
from concourse.bass_utils import run_bass_kernel_spmd
from concourse.tile import TileContext

# ---------------------------------------------------------------- constants
E = 500_000
F = 64
H = 128
N_OBS = 200_000
N_SAMP = 100_000
NCORES = 8

P = 128
OBS_PER_CORE = N_OBS // NCORES  # 25000 obs rows per core (int16-safe)
MASK_BANK = 32768               # known_mask bank size (int16-safe)
N_MASK_BANKS = math.ceil(N_SAMP / MASK_BANK)  # 4
GRP_CH = 8                      # chunks per gather group (1024 edges; ucode idx cap)
PAIR_CH = 8                     # chunks per compute pair-tile (1024 edges)
LOOK = 2                        # gather lookahead in groups

EM1 = float(np.exp(1.0) - 1.0)

FP = mybir.dt.float32
BF = mybir.dt.bfloat16
I16 = mybir.dt.int16
BFNP = ml_dtypes.bfloat16

_CACHE = {}

TRACE = False
LAST_RESULTS = None


def _build_nc(nch_cap, km_pieces):
    """Build the SPMD program.

    nch_cap: chunks (of 128 edges) per core, multiple of GRP_CH.
    km_pieces: list of (chunk_start, chunk_end, bank) with 128-aligned
        boundaries covering [0, nch_cap).
    """
    nc = bacc_mod.Bacc("TRN2", dynamic_dma_scratch_size=65536, num_swdge_queues=2)
    ncol = nch_cap * 8              # int16 idx columns (16 edges per column)
    e_cap = nch_cap * P
    T = PAIR_CH * P                 # 1024 edges per compute pair
    n_groups = nch_cap // GRP_CH
    n_pairs = n_groups * (GRP_CH // PAIR_CH)

    d_obs = nc.dram_tensor("obs_bf", [OBS_PER_CORE, H], BF, kind="ExternalInput")
    d_km = nc.dram_tensor("km_bf", [N_SAMP, 2 * F], BF, kind="ExternalInput")
    d_obs_idx = nc.dram_tensor("obs_idx16", [P, ncol], I16, kind="ExternalInput")
    d_km_idx = nc.dram_tensor("km_idx16", [P, ncol], I16, kind="ExternalInput")
    d_attr_pm = nc.dram_tensor("attr_pm", [P, nch_cap], FP, kind="ExternalInput")
    d_attr_bc = nc.dram_tensor("attr_bc", [P, e_cap // 2], BF, kind="ExternalInput")
    d_iota2c = nc.dram_tensor("iota2c", [P, 1], FP, kind="ExternalInput")
    d_fcbd = nc.dram_tensor("fcbd", [P, P], BF, kind="ExternalInput")
    d_w1lo = nc.dram_tensor("w1lo", [P, H], BF, kind="ExternalInput")
    d_w1hi = nc.dram_tensor("w1hi", [P, H], BF, kind="ExternalInput")
    d_w2 = nc.dram_tensor("w2", [H, F], BF, kind="ExternalInput")
    d_wrrlo = nc.dram_tensor("wrrlo", [P, H], BF, kind="ExternalInput")
    d_wrrhi = nc.dram_tensor("wrrhi", [P, H], BF, kind="ExternalInput")
    d_wrc = nc.dram_tensor("wrc", [H, H], BF, kind="ExternalInput")
    d_b1 = nc.dram_tensor("b_rm1", [H, 1], FP, kind="ExternalInput")
    d_b2p = nc.dram_tensor("b_rm2p", [P, 1], FP, kind="ExternalInput")
    d_brr = nc.dram_tensor("b_rr", [H, 1], FP, kind="ExternalInput")
    d_brc = nc.dram_tensor("b_rc", [H, 1], FP, kind="ExternalInput")
    d_ident = nc.dram_tensor("ident", [P, P], BF, kind="ExternalInput")
    d_iota = nc.dram_tensor("iota64", [P, F], BF, kind="ExternalInput")

    d_out = nc.dram_tensor("out_fm", [H, e_cap], BF, kind="ExternalOutput")

    gelu = mybir.ActivationFunctionType.Gelu
    mul = mybir.AluOpType.mult
    add = mybir.AluOpType.add
    neq = mybir.AluOpType.not_equal
    iseq = mybir.AluOpType.is_equal

    with TileContext(nc) as tc:
        with (
            tc.tile_pool(name="const", bufs=1) as cpool,
            tc.tile_pool(name="gather", bufs=6) as gpool,
            tc.tile_pool(name="work", bufs=4) as wpool,
            tc.tile_pool(name="ps", bufs=1, space="PSUM") as ps,
        ):
            obs_idx = cpool.tile_from(d_obs_idx[:, :])
            km_idx = cpool.tile_from(d_km_idx[:, :])

            groups = {}     # g -> dict of gather tiles
            pairs = {}      # p -> dict of stage tiles

            def emit_gathers(g):
                c0 = g * GRP_CH
                ge = GRP_CH * P
                g_obT = gpool.tile([P, 1, ge], BF, tag="g_obT", name=f"g_obT{g}")
                nc.gpsimd.dma_gather(
                    out_ap=g_obT[:, :, :],
                    in_ap=d_obs[:, :],
                    idxs_ap=obs_idx[:, c0 * 8:(c0 + GRP_CH) * 8],
                    num_idxs=ge, num_idxs_reg=ge,
                    elem_size=H, transpose=True, queue_num=0,
                    single_packet=False,
                )
                g_km = gpool.tile([P, GRP_CH, 2 * F], BF, tag="g_km", name=f"g_km{g}")
                for (ca, cb, bank) in km_pieces:
                    pa, pb = max(ca, c0), min(cb, c0 + GRP_CH)
                    if pa >= pb:
                        continue
                    blo = bank * MASK_BANK
                    bhi = min(blo + MASK_BANK, N_SAMP)
                    nc.gpsimd.dma_gather(
                        out_ap=g_km[:, pa - c0:pb - c0, :],
                        in_ap=d_km[blo:bhi, :],
                        idxs_ap=km_idx[:, pa * 8:pb * 8],
                        num_idxs=(pb - pa) * P, num_idxs_reg=(pb - pa) * P,
                        elem_size=2 * F, queue_num=1,
                    )
                abc = gpool.tile([P, GRP_CH * P // 2], BF, tag="abc", name=f"abc{g}")
                nc.sync.dma_start(
                    out=abc[:, :],
                    in_=d_attr_bc[:, c0 * P // 2:(c0 + GRP_CH) * P // 2],
                )
                groups[g] = {"obT": g_obT, "km": g_km, "abc": abc}

            def emit_s1(p):
                """softmax + q transpose + phi_rm layer 1."""
                g, t = divmod(p, GRP_CH // PAIR_CH)
                c0 = g * GRP_CH
                gg = groups[g]
                g_km = gg["km"]

                s4 = wpool.tile([P, PAIR_CH, F], BF, tag="s4", name=f"s4_{p}")
                ksum = wpool.tile([P, PAIR_CH], FP, tag="ksum", name=f"ks{p}")
                for j in range(PAIR_CH):
                    cj = t * PAIR_CH + j
                    nc.vector.scalar_tensor_tensor(
                        out=s4[:, j, :],
                        in0=iota64[:, :],
                        scalar=attr_pm[:, c0 + cj:c0 + cj + 1],
                        in1=g_km[:, cj, 0:F],
                        op0=neq, op1=mul,
                        accum_out=ksum[:, j:j + 1],
                    )
                dcol = wpool.tile([P, PAIR_CH], FP, tag="dcol", name=f"dc{p}")
                nc.vector.tensor_scalar(
                    out=dcol[:, :], in0=ksum[:, :],
                    scalar1=float(F), scalar2=None, op0=add,
                )
                rcol = wpool.tile([P, PAIR_CH], FP, tag="rcol", name=f"rc{p}")
                nc.vector.reciprocal_approx_fast(out=rcol[:, :], in_=dcol[:, :])
                q4 = wpool.tile([P, PAIR_CH, F], BF, tag="q4", name=f"q4_{p}")
                for j in range(PAIR_CH):
                    nc.vector.tensor_scalar(
                        out=q4[:, j, :], in0=s4[:, j, :],
                        scalar1=1.0, scalar2=rcol[:, j:j + 1],
                        op0=add, op1=mul,
                    )

                qt_ps = ps.tile([P, T // 2], BF, tag="qtp1", name=f"qt{p}",
                                padded_shape=[P, 2 * T])
                for j in range(PAIR_CH):
                    half, jj = divmod(j, PAIR_CH // 2)
                    nc.tensor.transpose(
                        out=qt_ps[half * F:(half + 1) * F, jj * P:(jj + 1) * P],
                        in_=q4[:, j, :],
                        identity=ident[:, :],
                        tile_position=(0, half * F),
                    )
                q_sb = wpool.tile([P, T // 2], BF, tag="q_sb", name=f"qs{p}")
                nc.vector.tensor_copy(out=q_sb[:, :], in_=qt_ps[:, :])

                p1 = ps.tile([H, T], FP, tag="qtp1", name=f"p1_{p}")
                nc.tensor.matmul(out=p1[:, 0:T // 2], lhsT=w1lo[:, :],
                                 rhs=q_sb[:, :], start=True, stop=True)
                nc.tensor.matmul(out=p1[:, T // 2:T], lhsT=w1hi[:, :],
                                 rhs=q_sb[:, :], start=True, stop=True)
                h1 = wpool.tile([H, T], BF, tag="h1", name=f"h1_{p}")
                nc.scalar.activation(out=h1[:, :], in_=p1[:, :], func=gelu,
                                     bias=b1[:, :])
                pairs[p] = {"h1": h1}

            def emit_s2(p):
                """phi_rm layer 2 + fea_corr product + phi_rr."""
                g, t = divmod(p, GRP_CH // PAIR_CH)
                gg = groups[g]
                pp = pairs[p]
                h1 = pp["h1"]

                p2 = ps.tile([P, T // 2], FP, tag="p2", name=f"p2_{p}")
                nc.tensor.matmul(out=p2[0:F, :], lhsT=w2[:, :],
                                 rhs=h1[:, 0:T // 2], start=True, stop=True,
                                 tile_position=(0, 0))
                nc.tensor.matmul(out=p2[F:P, :], lhsT=w2[:, :],
                                 rhs=h1[:, T // 2:T], start=True, stop=True,
                                 tile_position=(0, F))
                m2 = wpool.tile([P, T // 2], BF, tag="m2", name=f"m2_{p}")
                nc.scalar.activation(out=m2[:, :], in_=p2[:, :], func=gelu,
                                     bias=b2p[:, :])

                pcol = t * PAIR_CH * P // 2
                oh = wpool.tile([P, T // 2], BF, tag="oh", name=f"oh{p}")
                nc.vector.tensor_scalar(
                    out=oh[:, :], in0=gg["abc"][:, pcol:pcol + T // 2],
                    scalar1=iota2c[:, :], scalar2=None, op0=iseq,
                )
                aj = ps.tile([P, T // 2], FP, tag="aj", name=f"aj{p}")
                nc.tensor.matmul(out=aj[:, :], lhsT=fcbd[:, :],
                                 rhs=oh[:, :], start=True, stop=True)
                arr = wpool.tile([P, T // 2], BF, tag="arr", name=f"ar{p}")
                nc.vector.tensor_tensor(
                    out=arr[:, :], in0=aj[:, :], in1=m2[:, :], op=mul,
                )

                p3 = ps.tile([H, T], FP, tag="p3", name=f"p3_{p}")
                nc.tensor.matmul(out=p3[:, 0:T // 2], lhsT=wrrlo[:, :],
                                 rhs=arr[:, :], start=True, stop=True)
                nc.tensor.matmul(out=p3[:, T // 2:T], lhsT=wrrhi[:, :],
                                 rhs=arr[:, :], start=True, stop=True)
                a_t = wpool.tile([H, T], BF, tag="a_t", name=f"at{p}")
                nc.scalar.activation(out=a_t[:, :], in_=p3[:, :], func=gelu,
                                     bias=brr[:, :])
                pp["a_t"] = a_t

            def emit_s3(p):
                """obs product + phi_rc + output DMA."""
                g, t = divmod(p, GRP_CH // PAIR_CH)
                gg = groups[g]
                pp = pairs[p]
                tcol = t * T

                rcr = wpool.tile([H, T], BF, tag="rcr", name=f"rr{p}")
                nc.vector.tensor_tensor(
                    out=rcr[:, :], in0=pp["a_t"][:, :],
                    in1=gg["obT"][:, 0, tcol:tcol + T], op=mul,
                )
                p4 = ps.tile([H, T], FP, tag="p4", name=f"p4_{p}")
                nc.tensor.matmul(out=p4[:, 0:T // 2], lhsT=wrc[:, :],
                                 rhs=rcr[:, 0:T // 2], start=True, stop=True)
                nc.tensor.matmul(out=p4[:, T // 2:T], lhsT=wrc[:, :],
                                 rhs=rcr[:, T // 2:T], start=True, stop=True)
                out_sb = wpool.tile([H, T], BF, tag="out_sb", name=f"ob{p}")
                nc.scalar.activation(out=out_sb[:, :], in_=p4[:, :], func=gelu,
                                     bias=brc[:, :])
                base = (g * GRP_CH + t * PAIR_CH) * P
                nc.sync.dma_start(out=d_out[:, base:base + T], in_=out_sb[:, :])
                del pairs[p]

            pairs_per_grp = GRP_CH // PAIR_CH
            for g in range(min(LOOK, n_groups)):
                emit_gathers(g)
            # remaining consts load while the priming gathers run
            w1lo = cpool.tile_from(d_w1lo[:, :])
            w1hi = cpool.tile_from(d_w1hi[:, :])
            w2 = cpool.tile_from(d_w2[:, :])
            wrrlo = cpool.tile_from(d_wrrlo[:, :])
            wrrhi = cpool.tile_from(d_wrrhi[:, :])
            wrc = cpool.tile_from(d_wrc[:, :])
            b1 = cpool.tile_from(d_b1[:, :])
            b2p = cpool.tile_from(d_b2p[:, :])
            brr = cpool.tile_from(d_brr[:, :])
            brc = cpool.tile_from(d_brc[:, :])
            ident = cpool.tile_from(d_ident[:, :])
            iota64 = cpool.tile_from(d_iota[:, :])
            iota2c = cpool.tile_from(d_iota2c[:, :])
            fcbd = cpool.tile_from(d_fcbd[:, :])
            attr_pm = cpool.tile_from(d_attr_pm[:, :])
            for s in range(n_pairs + 2):
                if s % pairs_per_grp == 0:
                    gnext = s // pairs_per_grp + LOOK
                    if gnext < n_groups:
                        emit_gathers(gnext)
                if s - 2 >= 0:
                    emit_s3(s - 2)
                if 0 <= s - 1 < n_pairs:
                    emit_s2(s - 1)
                if s < n_pairs:
                    emit_s1(s)

    nc.finalize()
    return nc


def _roundup(x, m):
    return (x + m - 1) // m * m


def _wrap16(v):
    # idx16[p, s] = flat[s*16 + p]; 16-row block replicated to 128
    # partitions (one replica per Q7 core)
    blk = v.reshape(-1, 16).T
    return np.ascontiguousarray(np.tile(blk, (8, 1)))


def _marshal(inputs_np):
    """Host-side sharding/permutation. Returns (build_key, in_maps, perms)."""
    obs_idx = np.asarray(inputs_np["obs_idx"]).astype(np.int64)
    mask_idx = np.asarray(inputs_np["obs_mask_idx"]).astype(np.int64)
    attr_idx = np.asarray(inputs_np["attr_idx"]).astype(np.int64)

    core_of = obs_idx // OBS_PER_CORE
    counts = np.zeros((NCORES, N_MASK_BANKS), dtype=np.int64)
    per_core = []
    for c in range(NCORES):
        sel = np.nonzero(core_of == c)[0]
        banks = mask_idx[sel] // MASK_BANK
        order = np.argsort(banks, kind="stable")
        per_core.append(sel[order])
        counts[c] = np.bincount(banks, minlength=N_MASK_BANKS)

    seg_pad = [_roundup(int(counts[:, b].max()), P) for b in range(N_MASK_BANKS)]
    total = sum(seg_pad)
    total_cap = _roundup(total, GRP_CH * P)
    seg_pad[-1] += total_cap - total
    nch_cap = total_cap // P
    seg_starts = np.concatenate([[0], np.cumsum(seg_pad)]) // P  # chunk units
    km_pieces = tuple(
        (int(seg_starts[b]), int(seg_starts[b + 1]), b)
        for b in range(N_MASK_BANKS) if seg_starts[b + 1] > seg_starts[b]
    )

    f32 = np.float32
    km_bf = np.zeros((N_SAMP, 2 * F), dtype=BFNP)
    km_bf[:, 0:F] = (np.asarray(inputs_np["known_mask"], dtype=f32) * EM1)
    fcv = np.asarray(inputs_np["fea_corr"], dtype=f32)
    fcbd = np.zeros((P, P), dtype=BFNP)
    fcbd[0:F, 0:F] = fcv
    fcbd[F:P, F:P] = fcv
    iota2c = np.concatenate([np.arange(F, dtype=f32)] * 2).reshape(P, 1)

    w1 = np.asarray(inputs_np["W_rm1"], dtype=f32).astype(BFNP)      # [64,128]
    wrr = np.asarray(inputs_np["W_rr"], dtype=f32).astype(BFNP)      # [64,128]
    shared = {
        "km_bf": km_bf,
        "fcbd": fcbd,
        "iota2c": iota2c,
        "w1lo": np.vstack([w1, np.zeros((F, H), dtype=BFNP)]),
        "w1hi": np.vstack([np.zeros((F, H), dtype=BFNP), w1]),
        "w2": np.asarray(inputs_np["W_rm2"], dtype=f32).astype(BFNP),
        "wrrlo": np.vstack([wrr, np.zeros((F, H), dtype=BFNP)]),
        "wrrhi": np.vstack([np.zeros((F, H), dtype=BFNP), wrr]),
        "wrc": np.asarray(inputs_np["W_rc"], dtype=f32).astype(BFNP),
        "b_rm1": np.asarray(inputs_np["b_rm1"]).astype(f32).reshape(H, 1),
        "b_rm2p": np.tile(np.asarray(inputs_np["b_rm2"]).astype(f32), 2).reshape(P, 1),
        "b_rr": np.asarray(inputs_np["b_rr"]).astype(f32).reshape(H, 1),
        "b_rc": np.asarray(inputs_np["b_rc"]).astype(f32).reshape(H, 1),
        "ident": np.eye(P, dtype=BFNP),
        "iota64": np.tile(np.arange(F, dtype=f32), (P, 1)).astype(BFNP),
    }

    obs_embs = np.asarray(inputs_np["obs_embs"], dtype=f32).astype(BFNP)
    e_cap = nch_cap * P
    half = PAIR_CH * P // 2
    in_maps, perms = [], []
    for c in range(NCORES):
        ids = per_core[c]
        stream = np.full(e_cap, -1, dtype=np.int64)      # original edge ids
        obs_loc = np.zeros(e_cap, dtype=np.int16)
        km_loc = np.zeros(e_cap, dtype=np.int16)
        attr = np.zeros(e_cap, dtype=f32)
        pos = 0
        for b in range(N_MASK_BANKS):
            nb = int(counts[c, b])
            seg0 = int(seg_starts[b]) * P
            bank_ids = ids[pos:pos + nb]
            stream[seg0:seg0 + nb] = bank_ids
            obs_loc[seg0:seg0 + nb] = (obs_idx[bank_ids]
                                       - c * OBS_PER_CORE).astype(np.int16)
            km_loc[seg0:seg0 + nb] = (mask_idx[bank_ids]
                                      - b * MASK_BANK).astype(np.int16)
            attr[seg0:seg0 + nb] = attr_idx[bank_ids].astype(f32)
            pos += nb

        # pair-stacked attr broadcast: rows 0:64 = A-half cols, 64: = B-half
        attr_r = attr.reshape(-1, 2, half)
        attr_bc = np.empty((P, e_cap // 2), dtype=BFNP)
        attr_bc[0:F, :] = attr_r[:, 0, :].reshape(1, -1)
        attr_bc[F:P, :] = attr_r[:, 1, :].reshape(1, -1)

        in_maps.append({
            "obs_bf": obs_embs[c * OBS_PER_CORE:(c + 1) * OBS_PER_CORE],
            "obs_idx16": _wrap16(obs_loc),
            "km_idx16": _wrap16(km_loc),
            "attr_pm": np.ascontiguousarray(attr.reshape(nch_cap, P).T),
            "attr_bc": attr_bc,
            **shared,
        })
        perms.append(stream)

    return (nch_cap, km_pieces), in_maps, perms


def kernel(**inputs):
    global LAST_RESULTS
    inputs_np = {k: np.asarray(v) for k, v in inputs.items()}

    build_key, in_maps, perms = _marshal(inputs_np)
    if _CACHE.get("key") != build_key:
        _CACHE["nc"] = _build_nc(*build_key)
        _CACHE["key"] = build_key

    res = run_bass_kernel_spmd(
        _CACHE["nc"], in_maps, core_ids=list(range(NCORES)), trace=TRACE,
    )
    LAST_RESULTS = res

    out = np.empty((E, H), dtype=np.float32)
    for c in range(NCORES):
        out_fm = np.asarray(res.results[c]["out_fm"])    # [H, e_cap] bf16
        stream = perms[c]
        valid = stream >= 0
        out[stream[valid]] = out_fm.T[valid].astype(np.float32)
    return out
